# revision 1
# baseline (speedup 1.0000x reference)
# Multi-head attention (B=2, T=4096, DIM=1024, H=16, D=64) with RoPE,
# tensor-parallel over 8 TRN2 NeuronCores: core c handles batch c//4 and
# heads 4*(c%4) .. 4*(c%4)+3. Each core computes its 4 heads end-to-end and
# a partial output projection (row-parallel wo); the host sums the 4
# partials per batch and transposes back.
import numpy as np
import ml_dtypes

B, T, DIM = 2, 4096, 1024
HEADS, HD = 16, 64
N_CORES = 8
HPC = 4          # heads per core
JC = HPC * HD    # 256 projection cols per core
BF16 = ml_dtypes.bfloat16

_PROGRAM = None  # cached program


def _rope_tables_np():
    # matches reference.rope_tables(T, 64) in fp32
    inv_freq = 1.0 / (10000.0 ** (np.arange(0, HD, 2, dtype=np.float32) / HD))
    t = np.arange(T, dtype=np.float32)
    freqs = np.einsum("i,j->ij", t, inv_freq).astype(np.float32)  # [T, 32]
    emb = np.concatenate((freqs, freqs), axis=-1)  # [T, 64]
    cos = np.cos(emb).astype(np.float32)
    sin = np.sin(emb).astype(np.float32)
    # rotate_half: out[d] = q[d]*cos[d] + (-q[d+32] if d<32 else q[d-32])*sin[d]
    sgn = np.where(np.arange(HD) < 32, -1.0, 1.0).astype(np.float32)
    sinS = sin * sgn[None, :]
    return cos, sinS


def _build_program():
    """Build the SPMD Bass program (identical on all 8 cores)."""
    from concourse import bacc
    import concourse.mybir as mybir
    import concourse.tile as tile
    from concourse.masks import make_identity

    BF = mybir.dt.bfloat16
    F32 = mybir.dt.float32
    AF = mybir.ActivationFunctionType

    nc = bacc.Bacc("TRN2", debug=False, num_devices=N_CORES)

    xT = nc.dram_tensor("xT", [DIM, T], BF, kind="ExternalInput")
    wqT = nc.dram_tensor("wqT", [DIM, JC], BF, kind="ExternalInput")
    wkT = nc.dram_tensor("wkT", [DIM, JC], BF, kind="ExternalInput")
    wvT = nc.dram_tensor("wvT", [DIM, JC], BF, kind="ExternalInput")
    wo65 = nc.dram_tensor("wo65", [HD + 1, HPC, DIM], BF, kind="ExternalInput")
    cosn = nc.dram_tensor("cosn", [T, HD], F32, kind="ExternalInput")
    sinn = nc.dram_tensor("sinn", [T, HD], F32, kind="ExternalInput")
    # passthrough input: lets a benchmark chain executions back-to-back
    # (pout of step i fed as chain of step i+1) with no host/XLA transform
    chain = nc.dram_tensor("chain", [DIM, T], F32, kind="ExternalInput")
    pout = nc.dram_tensor("pout", [DIM, T], F32, kind="ExternalOutput")
    chk = nc.dram_tensor("chk", [1, 512], F32, kind="ExternalOutput")

    NCC = DIM // 128     # 8 contraction chunks
    NTB = T // 128       # 32 t-blocks of 128
    NSB = T // 128       # 32 s-blocks of 128
    NTW = T // 512       # 8 t-blocks of 512

    with tile.TileContext(nc) as tc:
        with (
            tc.tile_pool(name="const", bufs=1) as constp,
            tc.tile_pool(name="xp", bufs=2) as xp,
            tc.tile_pool(name="ropep", bufs=5) as ropep,
            tc.tile_pool(name="ptp", bufs=10) as ptp,
            tc.tile_pool(name="stagep", bufs=3) as stagep,
            tc.tile_pool(name="normp", bufs=3) as normp,
        ):
            # ---- persistent tiles ----
            ident = constp.tile([128, 128], BF)
            make_identity(nc, ident)

            # only wq is loaded ahead of the first x tile; wk/wv/tables/wo are
            # DMA'd after it (see tb==0 below) so the first projection matmul
            # is gated by just 768KB of queued DMA
            wq_s = constp.tile([128, NCC, JC], BF)
            nc.sync.dma_start(wq_s, wqT.ap().rearrange("(cc p) j -> p cc j", p=128))
            wk_s = constp.tile([128, NCC, JC], BF)
            wv_s = constp.tile([128, NCC, JC], BF)
            wo_s = constp.tile([HD + 1, HPC, DIM], BF)
            cos_s = constp.tile([128, NTB, HD], F32)
            sin_s = constp.tile([128, NTB, HD], F32)

            zbias = constp.tile([128, 1], F32)
            nc.vector.memset(zbias, 0.0)

            # chain passthrough (negligible: one small DMA in/out)
            chtile = constp.tile([1, 512], F32)
            nc.sync.dma_start(chtile, chain.ap()[0:1, 0:512])
            nc.sync.dma_start(chk.ap(), chtile)

            qTs = constp.tile([128, 2, T], BF)   # [j, t]: j = jb*128+p, head=j//64
            kTs = constp.tile([128, 2, T], BF)
            # v in normal layout per s-chunk; per head: col 0 = ones, 1..64 = d
            v_s = constp.tile([128, NSB, HPC * (HD + 1)], BF)
            yTs = constp.tile([HD + 1, HPC, T], BF)  # row 0 = junk (killed by wo zero row)

            v4 = v_s.rearrange("p sc (h u) -> p sc h u", h=HPC)
            for h in range(HPC):
                nc.vector.memset(v4[:, :, h, 0:1], 1.0)

            # ---- phase 1: projections + RoPE + transpose ----
            with (
                tc.tile_pool(name="psP", bufs=2, space="PSUM") as psP,
                tc.tile_pool(name="psT", bufs=3, space="PSUM") as psT,
            ):
                # q/k transposes are emitted one tb late so the in-order PE
                # isn't gated by the DVE RoPE chain (or, at tb0, the table
                # DMAs) right after each tb's projection matmuls
                trans_pending = []

                def emit_transposes(keep=0):
                    while len(trans_pending) > keep:
                        qr_, dstT_, tsl_ = trans_pending.pop(0)
                        for jb in range(2):
                            tp = psT.tile([128, 128], BF, tag="tp")
                            nc.tensor.transpose(
                                tp, qr_[:, jb * 128 : (jb + 1) * 128], ident
                            )
                            nc.vector.tensor_copy(dstT_[:, jb, tsl_], tp)

                for tb in range(NTB):
                    xt = xp.tile([128, NCC, 128], BF, tag="xt")
                    nc.sync.dma_start(
                        xt,
                        xT.ap().rearrange("(cc p) t -> p cc t", p=128)[
                            :, :, tb * 128 : (tb + 1) * 128
                        ],
                    )
                    if tb == 0:
                        nc.sync.dma_start(
                            wk_s, wkT.ap().rearrange("(cc p) j -> p cc j", p=128)
                        )
                        nc.sync.dma_start(
                            wv_s, wvT.ap().rearrange("(cc p) j -> p cc j", p=128)
                        )
                        nc.sync.dma_start(
                            cos_s, cosn.ap().rearrange("(tc p) d -> p tc d", p=128)
                        )
                        nc.sync.dma_start(
                            sin_s, sinn.ap().rearrange("(tc p) d -> p tc d", p=128)
                        )
                        nc.sync.dma_start(wo_s, wo65.ap())
                    tsl = slice(tb * 128, (tb + 1) * 128)

                    for wt, dstT in ((wq_s, qTs), (wk_s, kTs)):
                        P = psP.tile([128, JC], F32, tag="P")
                        for cc in range(NCC):
                            nc.tensor.matmul(
                                P, lhsT=xt[:, cc, :], rhs=wt[:, cc, :],
                                start=(cc == 0), stop=(cc == NCC - 1),
                            )
                        # RoPE: out = P*cos + swap(P)*sinS  (per 64-wide head)
                        A = ropep.tile([128, JC], F32, tag="A")
                        P4 = P.rearrange("p (h d) -> p h d", h=HPC)
                        ct = (
                            cos_s[:, tb, :]
                            .rearrange("p (o d) -> p o d", o=1)
                            .broadcast_to([128, HPC, HD])
                        )
                        nc.vector.tensor_mul(
                            A.rearrange("p (h d) -> p h d", h=HPC), P4, ct
                        )
                        Bt = ropep.tile([128, JC], F32, tag="B")
                        B4 = Bt.rearrange("p (h u d) -> p h u d", h=HPC, u=2)
                        P42 = P.rearrange("p (h u d) -> p h u d", h=HPC, u=2)
                        s0 = (
                            sin_s[:, tb, 0:32]
                            .rearrange("p (o d) -> p o d", o=1)
                            .broadcast_to([128, HPC, 32])
                        )
                        s1 = (
                            sin_s[:, tb, 32:64]
                            .rearrange("p (o d) -> p o d", o=1)
                            .broadcast_to([128, HPC, 32])
                        )
                        nc.vector.tensor_mul(B4[:, :, 0, :], P42[:, :, 1, :], s0)
                        nc.vector.tensor_mul(B4[:, :, 1, :], P42[:, :, 0, :], s1)
                        qr = ropep.tile([128, JC], BF, tag="qr")
                        nc.vector.tensor_add(qr, A, Bt)
                        trans_pending.append((qr, dstT, tsl))

                    V = psP.tile([128, JC], F32, tag="P")
                    for cc in range(NCC):
                        nc.tensor.matmul(
                            V, lhsT=xt[:, cc, :], rhs=wv_s[:, cc, :],
                            start=(cc == 0), stop=(cc == NCC - 1),
                        )
                    nc.vector.tensor_copy(
                        v4[:, tb, :, 1 : HD + 1],
                        V.rearrange("p (h d) -> p h d", h=HPC),
                    )
                    emit_transposes(keep=2)
                emit_transposes()

            # ---- phases 2+3 psum pools: scores pairs [128,1024] (2 banks x2),
            # and a shared 4-buf 1-bank pool for PV accumulators + out-proj ----
            with (
                tc.tile_pool(name="psS", bufs=2, space="PSUM") as psS,
                tc.tile_pool(name="psO", bufs=4, space="PSUM") as psO,
            ):
                # ---- phase 2: attention. Per (hp, tw, sb), both heads' score
                # tiles share one [128, 1024] psum tile so each ACT exp covers
                # both; QK runs as K=64 row-tiled matmuls (head A on PE rows
                # 0-63, head B on 64-127). The out-projection for each tw is
                # emitted as soon as its last head pair (hp=1) is normalized,
                # so PE's slack inside the ACT-bound attention phase absorbs
                # it instead of a serial tail. ----
                GRP = 8

                def emit_outproj(tw_, n):
                    # drip up to n column-blocks of tw_'s output projection
                    while outproj_pending:
                        if n <= 0:
                            return
                        n -= 1
                        cb = outproj_pending.pop(0)
                        cbsl = slice(cb * 128, (cb + 1) * 128)
                        osl = slice(tw_ * 512, (tw_ + 1) * 512)
                        po = psO.tile([128, 512], F32, tag="o")
                        for h in range(HPC):
                            nc.tensor.matmul(
                                po, lhsT=wo_s[:, h, cbsl], rhs=yTs[:, h, osl],
                                start=(h == 0), stop=(h == HPC - 1),
                            )
                        st = stagep.tile([128, 512], F32, tag="st")
                        nc.vector.tensor_copy(st, po)
                        nc.sync.dma_start(pout.ap()[cbsl, osl], st)

                outproj_pending = []
                for hp in range(2):
                    for tw in range(NTW):
                        twsl = slice(tw * 512, (tw + 1) * 512)
                        hA, hB = 2 * hp, 2 * hp + 1
                        oA = psO.tile([128, 512], F32, tag="o")
                        oB = psO.tile([128, 512], F32, tag="o")
                        for g in range(NSB // GRP):
                            pend = []
                            for i in range(GRP):
                                sb = g * GRP + i
                                ssl = slice(sb * 128, (sb + 1) * 128)
                                sAB = psS.tile([128, 1024], F32, tag="s")
                                nc.tensor.matmul(
                                    sAB[:, 0:512],
                                    lhsT=kTs[0:64, hp, ssl],
                                    rhs=qTs[0:64, hp, twsl],
                                    start=True, stop=True,
                                )
                                nc.tensor.matmul(
                                    sAB[:, 512:1024],
                                    lhsT=kTs[64:128, hp, ssl],
                                    rhs=qTs[64:128, hp, twsl],
                                    start=True, stop=True,
                                )
                                pAB = ptp.tile([128, 1024], BF, tag="pT")
                                nc.scalar.activation(
                                    pAB, sAB, AF.Exp, bias=zbias, scale=0.125
                                )
                                pend.append((sb, pAB))
                            # drip the previous window's out-projection AFTER
                            # this group's QK/exp so PE feeds ACT first and
                            # fills the drip during the exps
                            emit_outproj(tw - 1, 2)
                            for sb, pAB in pend:
                                nc.tensor.matmul(
                                    oA[0 : HD + 1, :],
                                    lhsT=v_s[:, sb, hA * 65 : hA * 65 + 65],
                                    rhs=pAB[:, 0:512],
                                    start=(sb == 0), stop=(sb == NSB - 1),
                                )
                                nc.tensor.matmul(
                                    oB[0 : HD + 1, :],
                                    lhsT=v_s[:, sb, hB * 65 : hB * 65 + 65],
                                    rhs=pAB[:, 512:1024],
                                    start=(sb == 0), stop=(sb == NSB - 1),
                                )
                        for o, h in ((oA, hA), (oB, hB)):
                            rc = normp.tile([1, 512], F32, tag="rc")
                            nc.vector.reciprocal(rc, o[0:1, :])
                            bc = normp.tile([HD + 1, 512], F32, tag="bc")
                            nc.gpsimd.partition_broadcast(bc, rc)
                            nc.vector.tensor_mul(
                                yTs[:, h, twsl], o[0 : HD + 1, :], bc
                            )
                        if hp == 1:
                            # queue this tw's out-projection (K=65; zero wo
                            # row kills the denominator row); it is dripped
                            # through the next tw's attention groups
                            emit_outproj(tw - 1, 8)  # drain any leftovers
                            outproj_pending = list(range(8))
                emit_outproj(NTW - 1, 8)  # final tw's out-projection

    nc.compile()
    return nc


def _get_program():
    global _PROGRAM
    if _PROGRAM is None:
        _PROGRAM = _build_program()
    return _PROGRAM


def make_in_maps(x, wq, wk, wv, wo):
    """Host-side sharding/layout prep: per-core input dicts."""
    x = np.asarray(x, dtype=np.float32)
    wq = np.asarray(wq, dtype=np.float32)
    wk = np.asarray(wk, dtype=np.float32)
    wv = np.asarray(wv, dtype=np.float32)
    wo = np.asarray(wo, dtype=np.float32)
    cos, sinS = _rope_tables_np()

    xT_b = [np.ascontiguousarray(x[b].T).astype(BF16) for b in range(B)]
    in_maps = []
    for c in range(N_CORES):
        b, hg = divmod(c, HPC)
        jsl = slice(hg * JC, (hg + 1) * JC)
        wqTc = np.ascontiguousarray(wq[jsl, :].T).astype(BF16)
        wkTc = np.ascontiguousarray(wk[jsl, :].T).astype(BF16)
        wvTc = np.ascontiguousarray(wv[jsl, :].T).astype(BF16)
        # wo65[0] = 0; wo65[1+d, h, co] = wo[co, hg*256 + h*64 + d]
        wo65 = np.zeros((HD + 1, HPC, DIM), dtype=np.float32)
        wo_cols = wo[:, jsl]  # [DIM, 256]
        wo65[1:, :, :] = wo_cols.reshape(DIM, HPC, HD).transpose(2, 1, 0)
        in_maps.append(
            {
                "xT": xT_b[b],
                "wqT": wqTc,
                "wkT": wkTc,
                "wvT": wvTc,
                "wo65": wo65.astype(BF16),
                "cosn": cos,
                "sinn": sinS,
                "chain": _ZCHAIN,
            }
        )
    return in_maps


_ZCHAIN = np.zeros((DIM, T), dtype=np.float32)


def assemble(results):
    """Host-side unshard: sum 4 head-group partials per batch, transpose."""
    out = np.zeros((B, T, DIM), dtype=np.float32)
    for b in range(B):
        acc = np.zeros((DIM, T), dtype=np.float32)
        for hg in range(HPC):
            acc += results[b * HPC + hg]["pout"]
        out[b] = acc.T
    return out


def kernel(x, wq, wk, wv, wo):
    from concourse.bass_utils import run_bass_kernel_spmd

    nc = _get_program()
    in_maps = make_in_maps(x, wq, wk, wv, wo)
    res = run_bass_kernel_spmd(nc, in_maps, core_ids=list(range(N_CORES)))
    return assemble(res.results)


if __name__ == "__main__":
    nc = _get_program()
    print("program built + compiled OK")



# revision 2
# speedup vs baseline: 1.3156x; 1.3156x over previous
# Multi-head attention (B=2, T=4096, DIM=1024, H=16, D=64) with RoPE,
# tensor-parallel over 8 TRN2 NeuronCores: core c handles batch c//4 and
# heads 4*(c%4) .. 4*(c%4)+3. Each core computes its 4 heads end-to-end and
# a partial output projection (row-parallel wo); the host sums the 4
# partials per batch and transposes back.
#
# v2: fused q+k projection matmuls (N=512), out-projection contracts head
# PAIRS at K=128 (no denominator ballast row), ACT preloads the exp table at
# t=0 and absorbs the phase-1 copies (transpose evacuation + v) that made
# DVE the projection-phase bottleneck.
import numpy as np
import ml_dtypes

B, T, DIM = 2, 4096, 1024
HEADS, HD = 16, 64
N_CORES = 8
HPC = 4          # heads per core
JC = HPC * HD    # 256 projection cols per core
BF16 = ml_dtypes.bfloat16

_PROGRAM = None  # cached program


def _rope_tables_np():
    # matches reference.rope_tables(T, 64) in fp32
    inv_freq = 1.0 / (10000.0 ** (np.arange(0, HD, 2, dtype=np.float32) / HD))
    t = np.arange(T, dtype=np.float32)
    freqs = np.einsum("i,j->ij", t, inv_freq).astype(np.float32)  # [T, 32]
    emb = np.concatenate((freqs, freqs), axis=-1)  # [T, 64]
    cos = np.cos(emb).astype(np.float32)
    sin = np.sin(emb).astype(np.float32)
    # rotate_half: out[d] = q[d]*cos[d] + (-q[d+32] if d<32 else q[d-32])*sin[d]
    sgn = np.where(np.arange(HD) < 32, -1.0, 1.0).astype(np.float32)
    sinS = sin * sgn[None, :]
    return cos, sinS


def _build_program():
    """Build the SPMD Bass program (identical on all 8 cores)."""
    from concourse import bacc
    import concourse.mybir as mybir
    import concourse.tile as tile
    from concourse.masks import make_identity

    BF = mybir.dt.bfloat16
    F32 = mybir.dt.float32
    I16 = mybir.dt.int16
    AF = mybir.ActivationFunctionType
    # bf16-domain Schraudolph fast-exp for exp(0.125*s): the int16 value
    # round(128/ln2 * 0.125*s + (127*128 - 8)) IS the bf16 bit pattern
    FEXP_A = float(0.125 * 128.0 / np.log(2.0))
    FEXP_B = float(127 * 128 - 8)

    nc = bacc.Bacc("TRN2", debug=False, num_devices=N_CORES)

    xT = nc.dram_tensor("xT", [DIM, T], BF, kind="ExternalInput")
    wqkT = nc.dram_tensor("wqkT", [DIM, 2 * JC], BF, kind="ExternalInput")
    wvT = nc.dram_tensor("wvT", [DIM, JC], BF, kind="ExternalInput")
    wo2 = nc.dram_tensor("wo2", [128, 2, DIM], BF, kind="ExternalInput")
    cosn = nc.dram_tensor("cosn", [T, HD], F32, kind="ExternalInput")
    sinn = nc.dram_tensor("sinn", [T, HD], F32, kind="ExternalInput")
    # passthrough input: lets a benchmark chain executions back-to-back
    # (pout of step i fed as chain of step i+1) with no host/XLA transform
    chain = nc.dram_tensor("chain", [DIM, T], F32, kind="ExternalInput")
    pout = nc.dram_tensor("pout", [DIM, T], F32, kind="ExternalOutput")
    chk = nc.dram_tensor("chk", [1, 512], F32, kind="ExternalOutput")

    NCC = DIM // 128     # 8 contraction chunks
    NTB = T // 128       # 32 t-blocks of 128
    NSB = T // 128       # 32 s-blocks of 128
    NTW = T // 512       # 8 t-blocks of 512

    with tile.TileContext(nc) as tc:
        with (
            tc.tile_pool(name="const", bufs=1) as constp,
            tc.tile_pool(name="xp", bufs=2) as xp,
            tc.tile_pool(name="ropep", bufs=5) as ropep,
            tc.tile_pool(name="ptp", bufs=19) as ptp,
            tc.tile_pool(name="stagep", bufs=3) as stagep,
            tc.tile_pool(name="normp", bufs=3) as normp,
        ):
            # ---- persistent tiles ----
            ident = constp.tile([128, 128], BF)
            make_identity(nc, ident)

            zbias = constp.tile([128, 1], F32)
            nc.vector.memset(zbias, 0.0)
            # preload the exp table set while projections run (one tiny exp)
            dummye = constp.tile([128, 1], F32)
            nc.scalar.activation(dummye, zbias, AF.Exp, bias=zbias, scale=1.0)

            # wqk loads right after the first x group (see the tb loop);
            # wv/tables/wo go on the Activation HWDGE queue so they never
            # delay the x stream on the SP queue
            wqk_s = constp.tile([128, NCC, 2 * JC], BF)
            wv_s = constp.tile([128, NCC, JC], BF)
            wo_s = constp.tile([128, 2, DIM], BF)
            cos_s = constp.tile([128, NTB, HD], F32)
            sin_s = constp.tile([128, NTB, HD], F32)

            # chain passthrough (negligible: one small DMA in/out)
            chtile = constp.tile([1, 512], F32)
            nc.sync.dma_start(chtile, chain.ap()[0:1, 0:512])
            nc.sync.dma_start(chk.ap(), chtile)

            qTs = constp.tile([128, 2, T], BF)   # [j, t]: j = jb*128+p, head=j//64
            kTs = constp.tile([128, 2, T], BF)
            # v in normal layout per s-chunk; per head: col 0 = ones, 1..64 = d
            v_s = constp.tile([128, NSB, HPC * (HD + 1)], BF)
            # normalized attention output, transposed, head-PAIRED for the
            # out-projection: y2[a*64+d, m, t] = y_head(2m+a)[d, t]
            yTs = constp.tile([128, 2, T], BF)

            v4 = v_s.rearrange("p sc (h u) -> p sc h u", h=HPC)
            for h in range(HPC):
                nc.vector.memset(v4[:, :, h, HD : HD + 1], 1.0)

            # ---- phase 1: projections + RoPE + transpose ----
            with (
                tc.tile_pool(name="psQK", bufs=2, space="PSUM") as psQK,
                tc.tile_pool(name="psV", bufs=2, space="PSUM") as psV,
                tc.tile_pool(name="psT", bufs=3, space="PSUM") as psT,
            ):
                # q/k transposes are emitted one tb late so the in-order PE
                # isn't gated by the DVE RoPE chain (or, at tb0, the table
                # DMAs) right after each tb's projection matmuls
                trans_pending = []

                def emit_transposes(keep=0):
                    while len(trans_pending) > keep:
                        qr_, dstT_, tsl_ = trans_pending.pop(0)
                        for jb in range(2):
                            tp = psT.tile([128, 128], BF, tag="tp")
                            nc.tensor.transpose(
                                tp, qr_[:, jb * 128 : (jb + 1) * 128], ident
                            )
                            # evacuate on ACT: it is idle during phase 1
                            nc.scalar.copy(dstT_[:, jb, tsl_], tp)

                for tb in range(NTB):
                    # fetch x in groups of 4 t-blocks: 1KB contiguous runs per
                    # (partition, cc) give ~4x better DMA efficiency than the
                    # 256B runs of a single t-block
                    if tb % 4 == 0:
                        xt4 = xp.tile([128, NCC, 512], BF, tag="xt")
                        nc.sync.dma_start(
                            xt4,
                            xT.ap().rearrange("(cc p) t -> p cc t", p=128)[
                                :, :, tb * 128 : (tb + 4) * 128
                            ],
                        )
                    xt = xt4[:, :, (tb % 4) * 128 : (tb % 4 + 1) * 128]
                    if tb == 0:
                        # wqk in halves so the first cc matmuls start sooner;
                        # then the tables needed by tb0's RoPE/V, wo deferred
                        wqk_src = wqkT.ap().rearrange("(cc p) j -> p cc j", p=128)
                        nc.sync.dma_start(wqk_s[:, 0:4, :], wqk_src[:, 0:4, :])
                        nc.sync.dma_start(wqk_s[:, 4:8, :], wqk_src[:, 4:8, :])
                        nc.sync.dma_start(
                            wv_s, wvT.ap().rearrange("(cc p) j -> p cc j", p=128)
                        )
                        nc.sync.dma_start(
                            cos_s, cosn.ap().rearrange("(tc p) d -> p tc d", p=128)
                        )
                        nc.sync.dma_start(
                            sin_s, sinn.ap().rearrange("(tc p) d -> p tc d", p=128)
                        )
                    if tb == 8:
                        # wo isn't needed until the attention phase
                        nc.sync.dma_start(wo_s, wo2.ap())
                    tsl = slice(tb * 128, (tb + 1) * 128)

                    # fused q|k projection: one N=512 matmul per cc chunk,
                    # plus v (N=256) sharing the same stationary xt chunk
                    P = psQK.tile([128, 2 * JC], F32, tag="qk")
                    V = psV.tile([128, JC], F32, tag="v")
                    for cc in range(NCC):
                        nc.tensor.matmul(
                            P, lhsT=xt[:, cc, :], rhs=wqk_s[:, cc, :],
                            start=(cc == 0), stop=(cc == NCC - 1),
                        )
                        nc.tensor.matmul(
                            V, lhsT=xt[:, cc, :], rhs=wv_s[:, cc, :],
                            start=(cc == 0), stop=(cc == NCC - 1),
                        )
                    # v evacuation on ACT (idle in phase 1)
                    nc.scalar.copy(
                        v4[:, tb, :, 0:HD],
                        V.rearrange("p (h d) -> p h d", h=HPC),
                    )
                    # RoPE: out = S*cos + swap(S)*sinS  (per 64-wide head)
                    for qk, dstT in ((0, qTs), (1, kTs)):
                        S = P[:, qk * JC : (qk + 1) * JC]
                        A = ropep.tile([128, JC], F32, tag="A")
                        S4 = S.rearrange("p (h d) -> p h d", h=HPC)
                        ct = (
                            cos_s[:, tb, :]
                            .rearrange("p (o d) -> p o d", o=1)
                            .broadcast_to([128, HPC, HD])
                        )
                        nc.vector.tensor_mul(
                            A.rearrange("p (h d) -> p h d", h=HPC), S4, ct
                        )
                        Bt = ropep.tile([128, JC], F32, tag="B")
                        B4 = Bt.rearrange("p (h u d) -> p h u d", h=HPC, u=2)
                        S42 = S.rearrange("p (h u d) -> p h u d", h=HPC, u=2)
                        s0 = (
                            sin_s[:, tb, 0:32]
                            .rearrange("p (o d) -> p o d", o=1)
                            .broadcast_to([128, HPC, 32])
                        )
                        s1 = (
                            sin_s[:, tb, 32:64]
                            .rearrange("p (o d) -> p o d", o=1)
                            .broadcast_to([128, HPC, 32])
                        )
                        nc.vector.tensor_mul(B4[:, :, 0, :], S42[:, :, 1, :], s0)
                        nc.vector.tensor_mul(B4[:, :, 1, :], S42[:, :, 0, :], s1)
                        qr = ropep.tile([128, JC], BF, tag="qr")
                        nc.vector.tensor_add(qr, A, Bt)
                        trans_pending.append((qr, dstT, tsl))
                    emit_transposes(keep=2)
                emit_transposes()

            # ---- phases 2+3 psum pools: scores pairs [128,1024] (2 banks x2),
            # and a shared 4-buf 1-bank pool for PV accumulators + out-proj ----
            with (
                tc.tile_pool(name="psS", bufs=2, space="PSUM") as psS,
                tc.tile_pool(name="psO", bufs=4, space="PSUM") as psO,
            ):
                # ---- phase 2: attention. Per (hp, tw, sb), both heads' score
                # tiles share one [128, 1024] psum tile so each ACT exp covers
                # both; QK runs as K=64 row-tiled matmuls (head A on PE rows
                # 0-63, head B on 64-127). The out-projection for each tw is
                # emitted as soon as its last head pair (hp=1) is normalized,
                # so PE's slack inside the ACT-bound attention phase absorbs
                # it instead of a serial tail. ----
                GRP = 16

                def emit_outproj(tw_, n, drain=False):
                    # drip up to n column-blocks of tw_'s output projection;
                    # each block contracts head pairs at K=128 (2 matmuls).
                    # In the final drain ACT is idle: split the psum
                    # evacuation and the pout DMAs across both engines/queues.
                    while outproj_pending:
                        if n <= 0:
                            return
                        n -= 1
                        cb = outproj_pending.pop(0)
                        cbsl = slice(cb * 128, (cb + 1) * 128)
                        osl = slice(tw_ * 512, (tw_ + 1) * 512)
                        po = psO.tile([128, 512], F32, tag="o")
                        for m in range(2):
                            nc.tensor.matmul(
                                po, lhsT=wo_s[:, m, cbsl], rhs=yTs[:, m, osl],
                                start=(m == 0), stop=(m == 1),
                            )
                        st = stagep.tile([128, 512], F32, tag="st")
                        if drain and cb % 2 == 0:
                            nc.scalar.copy(st, po)
                            nc.scalar.dma_start(pout.ap()[cbsl, osl], st)
                        else:
                            nc.vector.tensor_copy(st, po)
                            nc.sync.dma_start(pout.ap()[cbsl, osl], st)

                outproj_pending = []
                for hp in range(2):
                    for tw in range(NTW):
                        twsl = slice(tw * 512, (tw + 1) * 512)
                        hA, hB = 2 * hp, 2 * hp + 1
                        oA = psO.tile([HD + 1, 512], F32, tag="o")
                        oB = psO.tile([HD + 1, 512], F32, tag="o")
                        for g in range(NSB // GRP):
                            pend = []
                            for i in range(GRP):
                                sb = g * GRP + i
                                ssl = slice(sb * 128, (sb + 1) * 128)
                                sAB = psS.tile([128, 1024], F32, tag="s")
                                nc.tensor.matmul(
                                    sAB[:, 0:512],
                                    lhsT=kTs[0:64, hp, ssl],
                                    rhs=qTs[0:64, hp, twsl],
                                    start=True, stop=True,
                                )
                                nc.tensor.matmul(
                                    sAB[:, 512:1024],
                                    lhsT=kTs[64:128, hp, ssl],
                                    rhs=qTs[64:128, hp, twsl],
                                    start=True, stop=True,
                                )
                                pAB = ptp.tile([128, 1024], BF, tag="pT")
                                if sb % 3 == 1:
                                    # fast-exp on DVE (Schraudolph, bf16
                                    # bit-domain) to offload the ACT exp
                                    # bottleneck; bitcast int16->bf16 is free
                                    nc.vector.tensor_scalar(
                                        pAB.bitcast(I16), sAB, FEXP_A, FEXP_B,
                                        op0=mybir.AluOpType.mult,
                                        op1=mybir.AluOpType.add,
                                    )
                                else:
                                    nc.scalar.activation(
                                        pAB, sAB, AF.Exp, bias=zbias, scale=0.125
                                    )
                                pend.append((sb, pAB))
                            # drip the previous window's out-projection AFTER
                            # this group's QK/exp so PE feeds ACT first and
                            # fills the drip during the exps
                            emit_outproj(tw - 1, 4)
                            for sb, pAB in pend:
                                nc.tensor.matmul(
                                    oA[0 : HD + 1, :],
                                    lhsT=v_s[:, sb, hA * 65 : hA * 65 + 65],
                                    rhs=pAB[:, 0:512],
                                    start=(sb == 0), stop=(sb == NSB - 1),
                                )
                                nc.tensor.matmul(
                                    oB[0 : HD + 1, :],
                                    lhsT=v_s[:, sb, hB * 65 : hB * 65 + 65],
                                    rhs=pAB[:, 512:1024],
                                    start=(sb == 0), stop=(sb == NSB - 1),
                                )
                        for o, h in ((oA, hA), (oB, hB)):
                            a = h % 2
                            rc = normp.tile([1, 512], F32, tag="rc")
                            nc.vector.reciprocal(rc, o[HD : HD + 1, :])
                            bc = normp.tile([HD, 512], F32, tag="bc")
                            nc.gpsimd.partition_broadcast(bc, rc)
                            nc.vector.tensor_mul(
                                yTs[a * HD : (a + 1) * HD, hp, twsl],
                                o[0:HD, :],
                                bc,
                            )
                        if hp == 1:
                            # queue this tw's out-projection; it is dripped
                            # through the next tw's attention groups
                            emit_outproj(tw - 1, 8)  # drain any leftovers
                            outproj_pending = list(range(8))
                emit_outproj(NTW - 1, 8, drain=True)  # final tw's out-projection

    nc.compile()
    return nc


def _get_program():
    global _PROGRAM
    if _PROGRAM is None:
        _PROGRAM = _build_program()
    return _PROGRAM


def make_in_maps(x, wq, wk, wv, wo):
    """Host-side sharding/layout prep: per-core input dicts."""
    x = np.asarray(x, dtype=np.float32)
    wq = np.asarray(wq, dtype=np.float32)
    wk = np.asarray(wk, dtype=np.float32)
    wv = np.asarray(wv, dtype=np.float32)
    wo = np.asarray(wo, dtype=np.float32)
    cos, sinS = _rope_tables_np()

    xT_b = [np.ascontiguousarray(x[b].T).astype(BF16) for b in range(B)]
    in_maps = []
    for c in range(N_CORES):
        b, hg = divmod(c, HPC)
        jsl = slice(hg * JC, (hg + 1) * JC)
        wqTc = np.ascontiguousarray(wq[jsl, :].T)
        wkTc = np.ascontiguousarray(wk[jsl, :].T)
        wqkTc = np.concatenate([wqTc, wkTc], axis=1).astype(BF16)  # [DIM, 512]
        wvTc = np.ascontiguousarray(wv[jsl, :].T).astype(BF16)
        # wo2[a*64+d, m, co] = wo[co, hg*256 + m*128 + a*64 + d]
        wo_cols = wo[:, jsl]  # [DIM, 256]
        wo2 = np.ascontiguousarray(
            wo_cols.reshape(DIM, 2, 128).transpose(2, 1, 0)
        )
        in_maps.append(
            {
                "xT": xT_b[b],
                "wqkT": wqkTc,
                "wvT": wvTc,
                "wo2": wo2.astype(BF16),
                "cosn": cos,
                "sinn": sinS,
                "chain": _ZCHAIN,
            }
        )
    return in_maps


_ZCHAIN = np.zeros((DIM, T), dtype=np.float32)


def assemble(results):
    """Host-side unshard: sum 4 head-group partials per batch, transpose."""
    out = np.zeros((B, T, DIM), dtype=np.float32)
    for b in range(B):
        acc = np.zeros((DIM, T), dtype=np.float32)
        for hg in range(HPC):
            acc += results[b * HPC + hg]["pout"]
        out[b] = acc.T
    return out


def kernel(x, wq, wk, wv, wo):
    from concourse.bass_utils import run_bass_kernel_spmd

    nc = _get_program()
    in_maps = make_in_maps(x, wq, wk, wv, wo)
    res = run_bass_kernel_spmd(nc, in_maps, core_ids=list(range(N_CORES)))
    return assemble(res.results)


if __name__ == "__main__":
    nc = _get_program()
    print("program built + compiled OK")
